# revision 63
# baseline (speedup 1.0000x reference)
"""Trainium2 Bass kernel for the MultiHeadAttention problem (B=4,S=2048,D=1024,H=16).

Math implemented (matches reference.py, including its quirks):
  x = q  (k, v inputs ignored by the reference)
  Qh/Kh/Vh from x*proj_{Q,K,V}, head h uses channels [h*64,(h+1)*64)
  scores = Qh @ Kh^T / sqrt(D); mask index for (b,h) is (b*H+h) % B
  masked scores -> -1e-10, so exp(masked) == 1.0f exactly in fp32
  softmax numerator n_k = nm_k*exp(s_k) + m_k   (nm = 1-mask, m = mask)
  ctx = (sum_k n_k xv_k)/Z;  Z collected via xv's 65th all-ones column
  out = LayerNorm(ctx + q) * gamma + beta

Device decomposition per core (8 cores; core c -> batch b=c//2, query half
c%2), engineered so the ACT engine's exp stream (256 x [128,1024] psum->bf16,
~266us) is the sole pacer and every other engine hides under it:
  mm1:    scoresT[k,q] chunk = qtw-slice.T @ qtr-slice (bf16, psum, bufs=2)
  exp:    ACT Exp psum -> at[pc] (only ACT work in steady state; no table
          switch; Z column means no max-subtraction pass is needed)
  tt:     DVE at[pc] *= nm[pc]  (bf16 2x mode); nm = 1-m built on-core by a
          4x tensor_scalar (halves mask HBM traffic)
  mm2:    ctx[qb 128, 65] += at-chunk(stationary [128k,128q]) @ xv(moving 65)
          -- sim cost = moving cols only; kills the old transpose stage
  maskmm: ctx += m-chunk(stationary) @ xv  (the "+m_k" term; replaces both
          the baseline's DVE (exp-1) pass and its colsum rank-1 trick)
  norm:   rz = 1/ctx[:,64]; qr[qb] slice += ctx*rz (residual fused, f32)
  LN:     bn_stats/aggr + magic-rsqrt Newton on DVE; (x-mu)*y applied by ACT
          Identity(scale=y, bias=-mu*y); per-qb fused tail feeds out-DMA

Scheduling: one emission slot per (head, kc); PE stream per slot =
[mm1, maskmm, mm2-drain(lag 4)] so the in-order PE queue never starves ACT;
mm2 drains carry across head boundaries (ctx psum double-buffered, each ctx
tile padded to a full psum bank because a start=True matmul zeroes its whole
bank -- only the first region per bank starts, siblings accumulate onto the
bank-zero).  Head 0 defers maskmm by 8 slots and nm-builds by 4 so the
startup DMA (qtr0/qtw0 -> m(g0) -> xvg(g0) slab) stays off the critical
path; per-group xv slabs ([128, 16*260] single DMA) keep group prefetch off
the steady-state window.

Known env pitfalls baked in here: walrus accepts 1 sem wait/instr
(_split_multi_waits), TileContext drain needs the same fix
(_patch_tile_drain), and nc.gpsimd compute ops hang real HW (NRT_EXEC_UNIT_
UNRECOVERABLE) -- keep elementwise work on DVE/ACT only.
"""

import numpy as np
import ml_dtypes

bf16 = ml_dtypes.bfloat16
B, S, D, H, DH = 4, 2048, 1024, 16, 64
HALF = S // 2  # 1024 query rows per core
NCORES = 8
LN_EPS = 1e-5

_CACHE = {}


def _patch_tile_drain(tile, mybir, bass_rust):
    """The walrus build in this env allows only one sem wait per (non-event)
    instruction; TileContext's exit drain can collect several (one per
    outstanding DMA queue).  Spread them over a chain of single-wait drains."""
    if getattr(tile.TileContext, "_drain_patched", False):
        return

    def _patched(self, tick_clock, wait_clock):
        drain_inst = self.nc.sync.drain()
        wait_clock.add_sem_waits(
            drain_inst.ins, bass_rust.ScopedClock({None: tick_clock.global_clock})
        )
        ii = drain_inst.ins
        waits = list(ii.sync_info.on_wait) if ii.sync_info else []
        if len(waits) > 1:
            ii.sync_info = mybir.SyncInfo(on_wait=[waits[0]], on_update=[])
            for w in waits[1:]:
                extra = self.nc.sync.drain()
                extra.ins.sync_info = mybir.SyncInfo(on_wait=[w], on_update=[])
        self.nc.all_engine_barrier()
        popped = self.nc._tile_sem_poison_stack.pop()
        assert popped is self._sem_poison
        self.nc.clear_and_free_semaphores(list(self.sems.allocated().values()))
        self.nc.all_engine_barrier()

    tile.TileContext._drain_and_barrier = _patched
    tile.TileContext._drain_patched = True


def _split_multi_waits(nc, mybir):
    """This env's walrus accepts only one sync wait per instruction (two for
    EventSemaphore).  Hoist extra waits onto preceding same-engine NoOps —
    engines are in-order, so semantics are identical."""
    for f in nc.m.functions:
        for blk in f.blocks:
            out = []
            changed = False
            for inst in blk.instructions:
                si = inst.sync_info
                waits = list(si.on_wait) if si and si.on_wait else []
                limit = 2 if isinstance(inst, mybir.InstEventSemaphore) else 1
                if len(waits) > limit:
                    changed = True
                    for i, w in enumerate(waits[: len(waits) - limit]):
                        nop = mybir.InstNoOp(name=f"{inst.name}.w{i}", ins=[], outs=[])
                        nop.engine = inst.engine
                        nop.sync_info = mybir.SyncInfo(on_wait=[w], on_update=[])
                        out.append(nop)
                    inst.sync_info = mybir.SyncInfo(
                        on_wait=waits[len(waits) - limit :],
                        on_update=list(si.on_update) if si.on_update else [],
                    )
                out.append(inst)
            if changed:
                blk.instructions = out


def _build_nc():
    import concourse.bass as bass
    import concourse.mybir as mybir
    import concourse.tile as tile
    import bass_rust

    _patch_tile_drain(tile, mybir, bass_rust)

    f32 = mybir.dt.float32
    b16 = mybir.dt.bfloat16
    i32 = mybir.dt.int32
    EXP = mybir.ActivationFunctionType.Exp
    IDENT = mybir.ActivationFunctionType.Identity
    MULT = mybir.AluOpType.mult
    ADD = mybir.AluOpType.add
    SUB = mybir.AluOpType.subtract
    SHR = mybir.AluOpType.arith_shift_right

    nc = bass.Bass(dynamic_dma_scratch_size=2048)

    qTw = nc.dram_tensor("qTw", [D, S], b16, kind="ExternalInput")
    qTr = nc.dram_tensor("qTr", [D, HALF], b16, kind="ExternalInput")
    # xVg[g][p][kc*260 + i*65 + dd] = xv value for k=kc*128+p, head g+4i
    xVg = nc.dram_tensor("xVg", [4, 128, 16 * 260], b16, kind="ExternalInput")
    # kc-PAIR layout: mT[g][pc][p][0:1024]    = mask[g][q, k=256*pc+p]
    #                 mT[g][pc][p][1024:2048] = mask[g][q, k=256*pc+128+p]
    mT = nc.dram_tensor("mT", [4, 8, 128, 2 * HALF], b16, kind="ExternalInput")
    qres = nc.dram_tensor("qres", [HALF, D], f32, kind="ExternalInput")
    out = nc.dram_tensor("out", [HALF, D], b16, kind="ExternalOutput")

    with tile.TileContext(nc) as tc:
        with (
            tc.tile_pool(name="persist", bufs=1) as P,
            tc.tile_pool(name="nmp", bufs=1) as NM,
            tc.tile_pool(name="mmp", bufs=1) as MM,
            tc.tile_pool(name="xgp", bufs=3) as XG,
            tc.tile_pool(name="abuf", bufs=6) as AB,
            tc.tile_pool(name="small", bufs=8) as SM,
            tc.tile_pool(name="otp", bufs=8) as OT,
            tc.tile_pool(name="ps_s", bufs=2, space="PSUM") as PS,
            tc.tile_pool(name="ps_c", bufs=2, space="PSUM") as PC,
        ):
            # ---- persistent tiles / loads, emitted in need-by order.
            qtw = [None] * 8
            qtr = [None] * 8
            xv = [None] * 16
            qrt = [None] * 8
            nm_cur = {}
            m_cur = {}

            def load_qj(j):
                t = P.tile([128, S], b16, tag=f"qtw{j}", name=f"qtw{j}")
                nc.sync.dma_start(t[:], qTw[j * 128 : (j + 1) * 128, :])
                qtw[j] = t
                r = P.tile([128, HALF], b16, tag=f"qtr{j}", name=f"qtr{j}")
                nc.sync.dma_start(r[:], qTr[j * 128 : (j + 1) * 128, :])
                qtr[j] = r

            def load_group_pc(g, pc):
                u = MM.tile([128, 2 * HALF], b16, tag=f"m{pc}", name=f"m{g}_{pc}")
                nc.sync.dma_start(u[:], mT[g, pc, :, :])
                m_cur[pc] = u

            def build_nm(g, pc):
                # nm = 1 - m on DVE (4x mode) instead of a second HBM stream
                t = NM.tile([128, 2 * HALF], b16, tag=f"nm{pc}", name=f"nm{g}_{pc}")
                nc.vector.tensor_scalar(
                    t[:], m_cur[pc][:], -1.0, 1.0, op0=MULT, op1=ADD
                )
                nm_cur[pc] = t

            def load_xvg(g):
                t = XG.tile([128, 16 * 260], b16, tag="xg", name=f"xg{g}")
                for hh in range(4):
                    nc.sync.dma_start(
                        t[:, hh * 4 * 260 : (hh + 1) * 4 * 260],
                        xVg[g, :, hh * 4 * 260 : (hh + 1) * 4 * 260],
                    )
                for kc in range(16):
                    xv[kc] = t[:, kc * 260 : (kc + 1) * 260]

            # startup: head0 (j=0) needs qtw0/qtr0 at once; m/nm g0 pc and
            # xv kc feed slot kc; head idx1 = h4 (j=2) at ~26us; idx2 (j=4)
            # ~45us; idx3 (j=6) ~65us; odd j only from group g2 (~160us).
            # qtr0 first, qtw0 in column chunks: mm1(kc0) only needs the
            # first 512 columns + all of qtr0
            r0 = P.tile([128, HALF], b16, tag="qtr0", name="qtr0")
            nc.sync.dma_start(r0[:], qTr[0:128, :])
            qtr[0] = r0
            # two separate tiles: no cross-chunk dep, mm1(kc0) waits only
            # the first 1024 columns
            qtw0ab = []
            for cchunk in range(2):
                tc0 = P.tile([128, HALF], b16, tag=f"qtw0{cchunk}",
                             name=f"qtw0{cchunk}")
                nc.sync.dma_start(
                    tc0[:], qTw[0:128, cchunk * 1024 : (cchunk + 1) * 1024]
                )
                qtw0ab.append(tc0)
            for pc in range(4):
                load_group_pc(0, pc)
            load_xvg(0)
            load_qj(2)
            for pc in range(4, 8):
                load_group_pc(0, pc)
            for qb in range(8):
                t = P.tile([128, D], f32, tag=f"qr{qb}", name=f"qr{qb}")
                nc.sync.dma_start(t[:], qres[qb * 128 : (qb + 1) * 128, :])
                qrt[qb] = t
            load_qj(4)
            load_qj(6)
            for j in (1, 3, 5, 7):
                load_qj(j)

            magic2_t = P.tile([128, 2], i32, tag="magic")
            nc.vector.memset(magic2_t[:], 0x5F3759DF)

            # ctx psum: per head 2 tiles x [128, 4*65]; qb -> tile qb//4,
            # cols (qb%4)*65.  PC bufs=2 double-buffers across heads.
            heads = [g + 4 * i for g in range(4) for i in range(4)]
            NHEADS = len(heads)
            ctx_of = {}   # head idx -> (tiles, head)
            m_of = {}     # head idx -> mask tiles snapshot
            at_of = {}    # (head idx, pc) -> A tile
            xslc_of = {}  # head idx -> xv col slices

            def ctx_slice(hi, qb):
                return ctx_of[hi][qb // 4][:, (qb % 4) * 65 : (qb % 4) * 65 + 65]

            def mm2_chunk(hi, kc):
                h = heads[hi]
                pc, hf = kc // 2, kc % 2
                a = at_of[(hi, pc)]
                for qb in range(8):
                    nc.tensor.matmul(
                        ctx_slice(hi, qb),
                        a[:, hf * HALF + qb * 128 : hf * HALF + (qb + 1) * 128],
                        xslc_of[hi][kc],
                        start=(hi == 0 and kc == 0 and qb % 4 == 0),
                        stop=(kc == 15 and hi > 0),
                        skip_group_check=True,
                    )

            def tt_chunk(hi, pc):
                nc.vector.tensor_tensor(
                    at_of[(hi, pc)][:], at_of[(hi, pc)][:], nm_cur[pc][:], op=MULT
                )

            def tt_half(hi, pc, hf):
                a = at_of[(hi, pc)][:, hf * HALF : (hf + 1) * HALF]
                nc.vector.tensor_tensor(
                    a, a, nm_cur[pc][:, hf * HALF : (hf + 1) * HALF], op=MULT
                )

            def maskmm_chunk(hi, kc, start, stop):
                pc, hf = kc // 2, kc % 2
                for qb in range(8):
                    nc.tensor.matmul(
                        ctx_slice(hi, qb),
                        m_of[hi][pc][:, hf * HALF + qb * 128 : hf * HALF + (qb + 1) * 128],
                        xslc_of[hi][kc],
                        start=(start and qb % 4 == 0),
                        stop=stop,
                        skip_group_check=True,
                    )

            def norm_one(hi, qb):
                h = heads[hi]
                cs = ctx_slice(hi, qb)
                rz = SM.tile([128, 1], f32, tag="rz")
                nc.vector.reciprocal(rz[:], cs[:, 64:65])
                # qr[qb][:, h*64:+64] = ctx*rz + qr  (residual fused)
                sl = qrt[qb][:, h * 64 : (h + 1) * 64]
                nc.vector.scalar_tensor_tensor(
                    sl, cs[:, 0:64], rz[:], sl, op0=MULT, op1=ADD
                )

            def norm_head(hi):
                for qb in range(8):
                    norm_one(hi, qb)

            TOT = NHEADS * 16
            LAG = 5
            for s in range(TOT + LAG):
                hi, kc = s // 16, s % 16
                if hi < NHEADS:
                    h = heads[hi]
                    g = h % 4
                    j, po = h // 2, (h % 2) * 64
                    if kc == 0:
                        if hi % 4 == 0 and hi > 0:
                            load_xvg(g)
                            for pc in range(8):
                                load_group_pc(g, pc)
                        # one full psum bank per tile: a start=True matmul
                        # zeroes its whole bank, so tiles must be bank-aligned
                        ctx_of[hi] = [
                            PC.tile([128, 512], f32, tag=f"ctx{half}",
                                    name=f"ctx{h}_{half}")
                            for half in range(2)
                        ]
                        i4 = h // 4
                        xslc_of[hi] = [
                            xv[k][:, i4 * 65 : (i4 + 1) * 65] for k in range(16)
                        ]

                        m_of[hi] = dict(m_cur)
                    pc, hf = kc // 2, kc % 2
                    # mm1: scores^T chunk kc -> psum [128, 1024]
                    p = PS.tile([128, HALF], f32, tag="scores", name=f"s{h}_{kc}")
                    if j == 0:
                        qtwsl = qtw0ab[kc // 8][
                            po : po + 64, (kc % 8) * 128 : (kc % 8 + 1) * 128
                        ]
                    else:
                        qtwsl = qtw[j][po : po + 64, kc * 128 : (kc + 1) * 128]
                    for qt in range(2):
                        nc.tensor.matmul(
                            p[:, qt * 512 : (qt + 1) * 512],
                            qtwsl,
                            qtr[j][po : po + 64, qt * 512 : (qt + 1) * 512],
                            start=True,
                            stop=True,
                        )
                    # maskmm: ctx += m-chunk.T @ xv-chunk.  Only the FIRST
                    # matmul touching a psum bank uses start=True: its
                    # bank-zero clears the sibling qb regions, which then
                    # accumulate with start=False.  For head 0 the maskmm
                    # stream is deferred 8 slots (mask DMA is off the
                    # startup critical path); mm2 chunk 0 owns start.
                    if hi == 0:
                        if kc >= 8:
                            maskmm_chunk(0, kc - 8, False, False)
                    else:
                        maskmm_chunk(hi, kc, kc == 0, False)
                        if hi == 1 and kc < 8:
                            maskmm_chunk(0, kc + 8, False, kc == 7)
                # mm2 drain: lag 6 for head 0 (its tt stream is deferred
                # behind the mask DMA), uniform lag 4 after
                if s >= 6 and s - 6 < 16:
                    mm2_chunk(0, s - 6)
                c = s - LAG
                if 16 <= c < TOT:
                    mm2_chunk(c // 16, c % 16)
                if hi < NHEADS:
                    # exp into at[pc] half hf
                    if hf == 0:
                        at_of[(hi, pc)] = AB.tile(
                            [128, 2 * HALF], b16, tag="at", name=f"A{h}_{pc}"
                        )
                    nc.scalar.activation(
                        at_of[(hi, pc)][:, hf * HALF : (hf + 1) * HALF], p[:], EXP
                    )
                    if hi == 0:
                        # deferred 4 slots: nm[pcd] built as m-DMA lands
                        if hf == 1 and kc >= 5:
                            pcd = (kc - 5) // 2
                            build_nm(g, pcd)
                            tt_chunk(0, pcd)
                    else:
                        if hi == 1 and hf == 1 and kc in (1, 3):
                            pcd = 6 if kc == 1 else 7
                            build_nm(0, pcd)
                            tt_chunk(0, pcd)
                        if hi % 4 == 0 and hi > 0 and hf == 0:
                            build_nm(g, pc)
                        # per-half tt: half 0 masks while the pair's second
                        # exp is still running; mm2 deps refine per half
                        tt_half(hi, pc, hf)
                # norm of previous head once its chunk15 has drained
                if hi >= 1 and hi <= NHEADS and kc == 8:
                    norm_head(hi - 1)

            # ---- LayerNorm in place on qr tiles (residual already added).
            # Last head's norm is fused per-qb so qb0's output DMA starts
            # as early as possible.
            for qb in range(8):
                norm_one(NHEADS - 1, qb)
                ot = qrt[qb]
                st = SM.tile([128, 2, 6], f32, tag="st")
                nc.vector.bn_stats(st[:, 0, :], ot[:, 0:512])
                nc.vector.bn_stats(st[:, 1, :], ot[:, 512:1024])
                mv = SM.tile([128, 2], f32, tag="mv")
                nc.vector.bn_aggr(mv[:], st[:])
                negmu = SM.tile([128, 1], f32, tag="nmu")
                nc.vector.tensor_scalar_mul(negmu[:], mv[:, 0:1], -1.0)
                var = SM.tile([128, 1], f32, tag="var")
                nc.vector.tensor_scalar_add(var[:], mv[:, 1:2], LN_EPS)
                # y = rsqrt(var): bit-magic seed + Newton iterations
                t1 = SM.tile([128, 1], i32, tag="t1")
                nc.vector.tensor_scalar(t1[:], var[:].bitcast(i32), 1, None, op0=SHR)
                y = SM.tile([128, 1], f32, tag="y")
                nc.vector.tensor_tensor(y[:].bitcast(i32), magic2_t[:, 0:1], t1[:], op=SUB)
                t2 = SM.tile([128, 1], f32, tag="t2")
                for _ in range(1):
                    nc.vector.tensor_tensor(t2[:], y[:], y[:], op=MULT)
                    nc.vector.tensor_tensor(t2[:], t2[:], var[:], op=MULT)
                    nc.vector.tensor_scalar(t2[:], t2[:], -0.5, 1.5, op0=MULT, op1=ADD)
                    nc.vector.tensor_tensor(y[:], y[:], t2[:], op=MULT)
                # (ot - mu)*y on the idle ACT engine: Identity(ot*y + (-mu*y))
                nmy = SM.tile([128, 1], f32, tag="nmy")
                nc.vector.tensor_tensor(nmy[:], negmu[:], y[:], op=MULT)
                o16 = OT.tile([128, D], b16, tag="o16", name=f"o16_{qb}")
                nc.scalar.activation(
                    o16[:], ot[:], IDENT, bias=nmy[:], scale=y[:]
                )
                nc.sync.dma_start(out[qb * 128 : (qb + 1) * 128, :], o16[:])

    _split_multi_waits(nc, mybir)
    return nc


def _prep_inputs(q, masks, proj_Q, proj_K, proj_V):
    """Host-side shard prep. Returns list of 8 in_maps."""
    q = np.asarray(q, dtype=np.float32)
    masks = np.asarray(masks)
    w = (proj_Q.astype(np.float64) * proj_K.astype(np.float64) / np.sqrt(D)).astype(
        np.float32
    )

    # mT[g][k, q] = masks[g][q, k] -> [4, S(k), S(q)]; nm is built on-core
    mT_full = masks.transpose(0, 2, 1).astype(bf16)

    in_maps = []
    per_batch = {}
    for b in range(B):
        qT = np.ascontiguousarray(q[b].T)  # [D, S] f32
        qTw_a = (qT * w[:, None]).astype(bf16)
        # xvg[g][kc*128+p][i*65+dd] -> flat [4, 128, 16*260]
        xq = q[b] * proj_V[None, :]  # [S, D] f32
        xvg = np.ones((4, S, 4 * 65), dtype=np.float32)
        for g in range(4):
            for i in range(4):
                h = g + 4 * i
                xvg[g, :, i * 65 : i * 65 + 64] = xq[:, h * 64 : (h + 1) * 64]
        xvg = xvg.reshape(4, 16, 128, 260).transpose(0, 2, 1, 3)
        xvg = np.ascontiguousarray(xvg).reshape(4, 128, 16 * 260)
        per_batch[b] = (qT, qTw_a, xvg.astype(bf16))

    def kcpair(a):
        # [4, 2048(k), 1024(q)] -> [4, 8, 128, 2048] kc-pair layout
        a = a.reshape(4, 8, 2, 128, HALF)  # [g, pc, half, p, q]
        return np.ascontiguousarray(a.transpose(0, 1, 3, 2, 4)).reshape(
            4, 8, 128, 2 * HALF
        )

    for c in range(NCORES):
        b, qh = c // 2, c % 2
        sl = slice(qh * HALF, (qh + 1) * HALF)
        qT, qTw_a, xvg16 = per_batch[b]
        mm = kcpair(np.ascontiguousarray(mT_full[:, :, sl]))
        in_maps.append(
            {
                "qTw": qTw_a,
                "qTr": np.ascontiguousarray(qT[:, sl]).astype(bf16),
                "xVg": xvg16,
                "mT": mm,
                "qres": np.ascontiguousarray(q[b][sl, :]),
            }
        )
    return in_maps


def kernel(q, k, v, masks, proj_Q, proj_K, proj_V, gamma, beta):
    from concourse.bass_utils import run_bass_kernel_spmd

    if "nc" not in _CACHE:
        _CACHE["nc"] = _build_nc()
    nc = _CACHE["nc"]

    in_maps = _prep_inputs(q, masks, proj_Q, proj_K, proj_V)
    res = run_bass_kernel_spmd(nc, in_maps, core_ids=list(range(NCORES)))
    _CACHE["last_exec_time_ns"] = res.exec_time_ns

    full = np.empty((B, S, D), dtype=np.float32)
    for c in range(NCORES):
        b, qh = c // 2, c % 2
        full[b, qh * HALF : (qh + 1) * HALF, :] = res.results[c]["out"].astype(
            np.float32
        )

    # Device kernel computes plain LayerNorm; fold gamma/beta on host only if
    # they are nontrivial (reference setup uses gamma=1, beta=0).
    gamma = np.asarray(gamma, dtype=np.float32)
    beta = np.asarray(beta, dtype=np.float32)
    if not (np.all(gamma == 1.0) and np.all(beta == 0.0)):
        full = full * gamma[None, None, :] + beta[None, None, :]
    return full
